# revision 8
# baseline (speedup 1.0000x reference)
"""Trainium2 Bass kernel for nn_AssigmentLayer (8-core data-parallel).

Math (B=131072, T=30, F=10, MAX_LEN=30, K=10 shifts):
  x_c = inputs[:, 0, c] for c in {0,1};  rc_c[m] = x_c[m//30] * w_{c}[m%30]
  out[b, j, 2i+c] = rc_c[j*B + b - i]   (0 for negative index), i in [0,10)
  out[b, j, 20+t] = inputs[b, j, 2+t],  t in [0,8)

Sharding: batch dim b split contiguously across 8 cores (B8=16384 each).
Per core, for each j, the needed rc values form one contiguous segment
  seg[j,c][t] = rc_c[m_base_j + t],  m_base_j = j*B + s*B8 - 9
computed on-device as outer products (PE matmuls, K=2 trick folds the
mod-30 phase correction), stored in a persistent SBUF tile (60 rows).
The 10-shift expansion is 10 PE transpose-matmuls per output tile whose
lhsT access patterns are shifted slices of the segment rows.
"""

import os
import sys

import numpy as np

if "/opt/trn_rl_repo" not in sys.path:
    sys.path.insert(0, "/opt/trn_rl_repo")

B = 131072
T = 30
F = 10
NCORES = 8
B8 = B // NCORES            # 16384
TILE_P = 119                # output rows per sub-tile (window = 128)
QUAD = 4                    # sub-tiles per group (merged DMAs)
XJW = 644                   # x row width per (c, j, ab)
NBROW = 640                 # batches computed per (c, j) row (5 * 128)
SEGCOLS = 30 * NBROW        # 19200 floats written per segment row
SEGROW = SEGCOLS + 32       # allocated row width

_CACHE = {}


def _sub_tile_starts():
    """Start rows (within a core) of each 119-row sub-tile; last overlaps."""
    starts = []
    b0 = 0
    while b0 + TILE_P < B8:
        starts.append(b0)
        b0 += TILE_P
    starts.append(B8 - TILE_P)
    return starts  # 138 entries, last = 16265


def _build_nc():
    import concourse.bacc as bacc
    import concourse.tile as tile
    from concourse import mybir
    from contextlib import ExitStack

    f32 = mybir.dt.float32
    nc = bacc.Bacc("TRN2", target_bir_lowering=False, debug=False,
                   num_devices=NCORES)

    tail_in = nc.declare_dram_parameter("tail", [B8, T, 8], f32, isOutput=False)
    xj_in = nc.declare_dram_parameter("xj", [128, XJW], f32, isOutput=False)
    wab_in = nc.declare_dram_parameter("wab", [128, 960], f32, isOutput=False)
    id_in = nc.declare_dram_parameter("ident", [60, 60], f32, isOutput=False)
    out_ext = nc.declare_dram_parameter("out", [B8, T, 28], f32, isOutput=True)

    starts = _sub_tile_starts()
    groups = [starts[i:i + QUAD] for i in range(0, len(starts), QUAD)]

    with tile.TileContext(nc) as tc:
        with ExitStack() as ctx:
            const_pool = ctx.enter_context(tc.tile_pool(name="const", bufs=1))
            seg_pool = ctx.enter_context(tc.tile_pool(name="seg", bufs=1))
            ps1_pool = ctx.enter_context(
                tc.tile_pool(name="ps1", bufs=2, space="PSUM"))
            stg_pool = ctx.enter_context(tc.tile_pool(name="stg", bufs=3))
            ps2_pool = ctx.enter_context(
                tc.tile_pool(name="ps2", bufs=4, space="PSUM"))
            out_pool = ctx.enter_context(tc.tile_pool(name="outp", bufs=3))
            tailp = ctx.enter_context(tc.tile_pool(name="tailp", bufs=3))

            # ---- constants / inputs resident in SBUF ----
            xall = const_pool.tile([128, XJW], f32)
            nc.scalar.dma_start(xall[:], xj_in[:])
            wblk = const_pool.tile([128, 960], f32)
            nc.scalar.dma_start(wblk[:], wab_in[:])
            ident = const_pool.tile([60, 60], f32)
            nc.scalar.dma_start(ident[:], id_in[:])

            # persistent segment rows: seg[2j+c, t] = rc_c[m_base_j + t]
            segsb = seg_pool.tile([60, SEGROW], f32)
            # staging: per partition p (batch 5p+ch), col r*150 + ch*30 + mt
            staging = seg_pool.tile([128, 9760], f32)

            # ---- stage 1: block-diagonal outer products -> staging ----
            # xall row 2r+ab (r = 2j+c), wblk[2r+ab, (r-32*g2)*30+mt]
            xr5 = xall[:, 0:NBROW].rearrange("p (a s) -> p s a", s=5)
            for ch in range(5):
                for g2 in range(2):
                    for h in range(2):
                        ps1 = ps1_pool.tile([128, 480], f32, tag="ps1")
                        nc.tensor.matmul(
                            ps1[:],
                            lhsT=xr5[64 * g2:64 * g2 + 64, ch, :],
                            rhs=wblk[64 * g2:64 * g2 + 64,
                                     480 * h:480 * h + 480],
                            start=True, stop=True,
                        )
                        r0 = 32 * g2 + 16 * h
                        off = r0 * 150 + ch * 30
                        dst = staging[:, off:off + 2400].rearrange(
                            "p (r m) -> p r m", m=150)[:, :, 0:30]
                        nc.vector.tensor_copy(
                            dst, ps1[:].rearrange("p (r m) -> p r m", r=16))
            # relayout: staging -> segment rows (SBUF->SBUF DMA)
            for r in range(60):
                dst = segsb[r:r + 1, 0:SEGCOLS]
                dst = dst.rearrange("p (q t) -> p q t", q=128)
                nc.scalar.dma_start(dst, staging[:, r * 150:(r + 1) * 150])

            # ---- stage 2: shift expansion + tail merge + store ----
            for grp in groups:
                ng = len(grp)
                otile = out_pool.tile([128, 840 * QUAD], f32, tag="otile")
                tstg = tailp.tile([128, 240 * QUAD], f32, tag="tstg")
                # merged tail load: rows b0v + p for each sub-tile v
                if all(grp[v] - grp[0] == v * TILE_P for v in range(ng)):
                    src = tail_in[grp[0]:grp[0] + (ng - 1) * TILE_P + TILE_P]
                    src = src.rearrange("(v p) j t -> p v (j t)", v=ng)
                    dst = tstg[0:TILE_P, 0:240 * ng].rearrange(
                        "p (v f) -> p v f", v=ng)
                    nc.gpsimd.dma_start(dst, src)
                else:
                    for v, b0 in enumerate(grp):
                        src = tail_in[b0:b0 + TILE_P]
                        src = src.rearrange("p j t -> p (j t)")
                        nc.gpsimd.dma_start(
                            tstg[0:TILE_P, 240 * v:240 * (v + 1)], src)
                for v, b0 in enumerate(grp):
                    psA = ps2_pool.tile([128, 300], f32, tag="ps2")
                    psB = ps2_pool.tile([128, 300], f32, tag="ps2")
                    for i in range(10):
                        ps = psA if i < 5 else psB
                        col = (i % 5) * 60
                        nc.tensor.transpose(
                            ps[0:TILE_P, col:col + 60],
                            segsb[:, b0 + 9 - i: b0 + 9 - i + TILE_P],
                            ident[:],
                        )
                    ovw = otile[0:TILE_P, 840 * v:840 * (v + 1)]
                    od = ovw.rearrange("p (j i c) -> p j i c", j=30, i=14, c=2)
                    for half, ps in ((0, psA), (1, psB)):
                        src = ps[0:TILE_P, :].rearrange(
                            "p (i j c) -> p j i c", i=5, j=30, c=2)
                        nc.vector.tensor_copy(
                            od[:, :, 5 * half:5 * half + 5, :], src)
                    # tail interleave
                    ts = tstg[0:TILE_P, 240 * v:240 * (v + 1)]
                    nc.scalar.copy(
                        ovw.rearrange("p (j k) -> p j k", j=30)[:, :, 20:28],
                        ts.rearrange("p (j t) -> p j t", j=30),
                    )
                # merged store
                if all(grp[v] - grp[0] == v * TILE_P for v in range(ng)):
                    dst = out_ext[grp[0]:grp[0] + (ng - 1) * TILE_P + TILE_P]
                    dst = dst.rearrange("(v p) j k -> p v (j k)", v=ng)
                    src = otile[0:TILE_P, 0:840 * ng].rearrange(
                        "p (v f) -> p v f", v=ng)
                    nc.gpsimd.dma_start(dst, src)
                else:
                    for v, b0 in enumerate(grp):
                        dst = out_ext[b0:b0 + TILE_P]
                        dst = dst.rearrange("p j k -> p (j k)")
                        nc.gpsimd.dma_start(
                            dst, otile[0:TILE_P, 840 * v:840 * (v + 1)])

    nc.compile()
    return nc


def _get_nc():
    if "nc" not in _CACHE:
        _CACHE["nc"] = _build_nc()
    return _CACHE["nc"]


def _prep_core(inputs, w1, w2, s):
    """Build the per-core input map (pure gather/layout, no arithmetic)."""
    f32 = np.float32
    x01 = inputs[:, 0, 0:2]                     # (B, 2)
    PAD = 4
    xpad = np.zeros((PAD + B + XJW + 8, 2), dtype=f32)
    xpad[PAD:PAD + B] = x01
    xj = np.zeros((128, XJW), dtype=f32)
    wab = np.zeros((128, 960), dtype=f32)
    w = [np.asarray(w1, f32).reshape(30), np.asarray(w2, f32).reshape(30)]
    e = np.arange(30)
    for c in range(2):
        for j in range(T):
            m_base = j * B + s * B8 - 9
            mb0 = m_base // 30
            o = m_base - 30 * mb0
            r = 2 * j + c                       # segment row
            row = 2 * r                         # xj/wab row for ab=0
            xj[row + 0] = xpad[PAD + mb0: PAD + mb0 + XJW, c]
            xj[row + 1] = xpad[PAD + mb0 + 1: PAD + mb0 + 1 + XJW, c]
            wa = w[c][(o + e) % 30].copy()
            wb = wa.copy()
            wa[o + e >= 30] = 0.0
            wb[o + e < 30] = 0.0
            g2 = r // 32
            col = (r - 32 * g2) * 30
            wab[row + 0, col:col + 30] = wa
            wab[row + 1, col:col + 30] = wb
    tail = np.ascontiguousarray(inputs[s * B8:(s + 1) * B8, :, 2:])
    return {
        "tail": tail,
        "xj": xj,
        "wab": wab,
        "ident": np.eye(60, dtype=f32),
    }


def _run(inputs, w1, w2, trace=False, trace_kwargs=None):
    from concourse.bass_utils import run_bass_kernel_spmd

    nc = _get_nc()
    inputs = np.asarray(inputs, dtype=np.float32)
    in_maps = [_prep_core(inputs, w1, w2, s) for s in range(NCORES)]
    res = run_bass_kernel_spmd(
        nc, in_maps, core_ids=list(range(NCORES)), trace=trace,
        **(trace_kwargs or {}),
    )
    out = np.concatenate(
        [res.results[i]["out"] for i in range(NCORES)], axis=0)
    return out, res


def kernel(inputs, w1, w2):
    return _run(inputs, w1, w2)[0]


# revision 9
# speedup vs baseline: 3.4646x; 3.4646x over previous
"""Trainium2 Bass kernel for nn_AssigmentLayer (8-core data-parallel).

Math (B=131072, T=30, F=10, MAX_LEN=30, K=10 shifts):
  x_c = inputs[:, 0, c] for c in {0,1};  rc_c[m] = x_c[m//30] * w_c[m%30]
  out[b, j, 2i+c] = rc_c[j*B + b - i]   (0 for negative index), i in [0,10)
  out[b, j, 20+t] = inputs[b, j, 2+t],  t in [0,8)

Sharding: batch dim b split contiguously across 8 cores (B8=16384 each).
Per core, for each (j, c), the needed rc values form one contiguous
segment seg[r=2j+c][t] = rc_c[m_base_j + t], m_base_j = j*B + s*B8 - 9.
The host passes index-gathered (no arithmetic) operand streams
  xs[r, t] = x_c[(m_base+t)//30],  ws[r, t] = w_c[(m_base+t)%30]
and the device computes seg = xs * ws (the actual multiplies), keeping
the 60 segment rows resident in SBUF. The 10-shift expansion is 10 PE
transpose-matmuls per 119-row output tile whose lhsT access patterns
are shifted slices of the segment rows; tail features ride along via
strided copies, and full 3360B-contiguous rows DMA out.
"""

import sys

import numpy as np

if "/opt/trn_rl_repo" not in sys.path:
    sys.path.insert(0, "/opt/trn_rl_repo")

B = 131072
T = 30
NCORES = 8
B8 = B // NCORES            # 16384
TILE_P = 119                # output rows per sub-tile (window = 128)
GRP = 6                     # sub-tiles per group (138 = 23*6)
SEGW = 16464                # segment row width (= 6 * 2744)
NCHUNK = 6
CHW = SEGW // NCHUNK        # 2744

_CACHE = {}


def _sub_tile_starts():
    starts = []
    b0 = 0
    while b0 + TILE_P < B8:
        starts.append(b0)
        b0 += TILE_P
    starts.append(B8 - TILE_P)
    return starts  # 138 entries, last = 16265


def _build_nc():
    import concourse.bacc as bacc
    import concourse.tile as tile
    from concourse import mybir
    from contextlib import ExitStack

    f32 = mybir.dt.float32
    nc = bacc.Bacc("TRN2", target_bir_lowering=False, debug=False,
                   num_devices=NCORES)

    tail_in = nc.declare_dram_parameter("tail", [B8, T, 8], f32, isOutput=False)
    xs_in = nc.declare_dram_parameter("xs", [60, SEGW], f32, isOutput=False)
    ws_in = nc.declare_dram_parameter("ws", [60, SEGW], f32, isOutput=False)
    id_in = nc.declare_dram_parameter("ident", [60, 60], f32, isOutput=False)
    out_ext = nc.declare_dram_parameter("out", [B8, T, 28], f32, isOutput=True)

    starts = _sub_tile_starts()
    groups = [starts[i:i + GRP] for i in range(0, len(starts), GRP)]

    with tile.TileContext(nc) as tc:
        with ExitStack() as ctx:
            const_pool = ctx.enter_context(tc.tile_pool(name="const", bufs=1))
            seg_pool = ctx.enter_context(tc.tile_pool(name="seg", bufs=1))
            xw_pool = ctx.enter_context(tc.tile_pool(name="xw", bufs=2))
            ps2_pool = ctx.enter_context(
                tc.tile_pool(name="ps2", bufs=8, space="PSUM"))
            out_pool = ctx.enter_context(tc.tile_pool(name="outp", bufs=3))
            tailp = ctx.enter_context(tc.tile_pool(name="tailp", bufs=3))

            ident = const_pool.tile([60, 60], f32)
            nc.scalar.dma_start(ident[:], id_in[:])

            # persistent segment rows: seg[2j+c, t] = rc_c[m_base_j + t]
            segsb = seg_pool.tile([60, SEGW], f32)

            # ---- stage 1: seg = xs * ws (chunked) ----
            for k in range(NCHUNK):
                xt = xw_pool.tile([60, CHW], f32, tag="xt")
                nc.scalar.dma_start(xt[:], xs_in[:, k * CHW:(k + 1) * CHW])
                wt = xw_pool.tile([60, CHW], f32, tag="wt")
                nc.scalar.dma_start(wt[:], ws_in[:, k * CHW:(k + 1) * CHW])
                nc.vector.tensor_mul(
                    segsb[:, k * CHW:(k + 1) * CHW], xt[:], wt[:])

            # ---- stage 2: shift expansion + tail merge + store ----
            for grp in groups:
                ng = len(grp)
                uniform = all(grp[v] - grp[0] == v * TILE_P for v in range(ng))
                otile = out_pool.tile([128, 840 * GRP], f32, tag="otile")
                tstg = tailp.tile([128, 240 * GRP], f32, tag="tstg")
                if uniform:
                    src = tail_in[grp[0]:grp[0] + ng * TILE_P]
                    src = src.rearrange("(v p) j t -> p v (j t)", v=ng)
                    dst = tstg[0:TILE_P, 0:240 * ng].rearrange(
                        "p (v f) -> p v f", v=ng)
                    nc.gpsimd.dma_start(dst, src)
                else:
                    for v, b0 in enumerate(grp):
                        src = tail_in[b0:b0 + TILE_P].rearrange(
                            "p j t -> p (j t)")
                        nc.gpsimd.dma_start(
                            tstg[0:TILE_P, 240 * v:240 * (v + 1)], src)
                for v, b0 in enumerate(grp):
                    psA = ps2_pool.tile([128, 300], f32, tag="ps2")
                    psB = ps2_pool.tile([128, 300], f32, tag="ps2")
                    for i in range(10):
                        ps = psA if i < 5 else psB
                        col = (i % 5) * 60
                        nc.tensor.transpose(
                            ps[0:TILE_P, col:col + 60],
                            segsb[:, b0 + 9 - i: b0 + 9 - i + TILE_P],
                            ident[:],
                        )
                    ovw = otile[0:TILE_P, 840 * v:840 * (v + 1)]
                    od = ovw.rearrange("p (j i c) -> p j i c", j=30, i=14, c=2)
                    srcA = psA[0:TILE_P, :].rearrange(
                        "p (i j c) -> p j i c", i=5, j=30, c=2)
                    nc.vector.tensor_copy(od[:, :, 0:5, :], srcA)
                    srcB = psB[0:TILE_P, :].rearrange(
                        "p (i j c) -> p j i c", i=5, j=30, c=2)
                    nc.scalar.copy(od[:, :, 5:10, :], srcB)
                    # tail interleave (alternate engines)
                    ts = tstg[0:TILE_P, 240 * v:240 * (v + 1)]
                    teng = nc.vector.tensor_copy if v % 2 == 0 else \
                        nc.scalar.copy
                    teng(
                        ovw.rearrange("p (j k) -> p j k", j=30)[:, :, 20:28],
                        ts.rearrange("p (j t) -> p j t", j=30),
                    )
                if uniform:
                    dst = out_ext[grp[0]:grp[0] + ng * TILE_P]
                    dst = dst.rearrange("(v p) j k -> p v (j k)", v=ng)
                    src = otile[0:TILE_P, 0:840 * ng].rearrange(
                        "p (v f) -> p v f", v=ng)
                    nc.gpsimd.dma_start(dst, src)
                else:
                    for v, b0 in enumerate(grp):
                        dst = out_ext[b0:b0 + TILE_P].rearrange(
                            "p j k -> p (j k)")
                        nc.gpsimd.dma_start(
                            dst, otile[0:TILE_P, 840 * v:840 * (v + 1)])

    nc.compile()
    return nc


def _get_nc():
    if "nc" not in _CACHE:
        _CACHE["nc"] = _build_nc()
    return _CACHE["nc"]


def _prep_core(inputs, w1, w2, s):
    """Per-core input map: pure index gathers, no arithmetic."""
    f32 = np.float32
    x01 = inputs[:, 0, 0:2]                     # (B, 2)
    PAD = 2
    NB = SEGW // 30 + 2                         # 550 batches per row
    xpad = np.zeros((PAD + B + NB + 4, 2), dtype=f32)
    xpad[PAD:PAD + B] = x01
    xs = np.empty((60, SEGW), dtype=f32)
    ws = np.empty((60, SEGW), dtype=f32)
    w = [np.asarray(w1, f32).reshape(T), np.asarray(w2, f32).reshape(T)]
    wtiled = [np.tile(w[c], NB + 1) for c in range(2)]
    for c in range(2):
        for j in range(T):
            m_base = j * B + s * B8 - 9
            mb0 = m_base // 30
            o = m_base - 30 * mb0
            r = 2 * j + c
            xs[r] = np.repeat(
                xpad[PAD + mb0:PAD + mb0 + NB, c], 30)[o:o + SEGW]
            ws[r] = wtiled[c][o:o + SEGW]
    tail = np.ascontiguousarray(inputs[s * B8:(s + 1) * B8, :, 2:])
    return {
        "tail": tail,
        "xs": xs,
        "ws": ws,
        "ident": np.eye(60, dtype=f32),
    }


def _run(inputs, w1, w2, trace=False, trace_kwargs=None):
    from concourse.bass_utils import run_bass_kernel_spmd

    nc = _get_nc()
    inputs = np.asarray(inputs, dtype=np.float32)
    in_maps = [_prep_core(inputs, w1, w2, s) for s in range(NCORES)]
    res = run_bass_kernel_spmd(
        nc, in_maps, core_ids=list(range(NCORES)), trace=trace,
        **(trace_kwargs or {}),
    )
    out = np.concatenate(
        [res.results[i]["out"] for i in range(NCORES)], axis=0)
    return out, res


def kernel(inputs, w1, w2):
    return _run(inputs, w1, w2)[0]
